# revision 46
# baseline (speedup 1.0000x reference)
"""Trainium2 Bass kernel for nn_Bezier (quadratic Bezier curve rasterization).

Reference semantics: 65536 curve samples, each scatter-adds a 32x32 truncated
Gaussian patch exp(-((x-ci)^2+(y-cj)^2)/(2*sigma^2)) into a 2048x2048 grid at
block corner (bx,by) = clip(floor(2048*curve)-16, 0, 2016); output is the
mean over samples.

Device algorithm (8 NeuronCores, SPMD), v2:
  The patch is separable (outer product of two 32-vectors), so each block of
  128 consecutive samples becomes one TensorE matmul contracting over the
  samples:  window[48x48] += SX.T @ SY,  where SX[k, i] is sample k's masked
  Gaussian strip over a 48-wide x-window and SY[k, j] the y-strip.  Two
  consecutive blocks (256 samples, coordinate drift <= 16 px guaranteed by
  |B'| <= 2) share one window and accumulate in PSUM.

  Strips are built without any per-sample tables:
    exponent T[k,i] = -INV*(x'_k - c'_i)^2 expands into a rank-3 bilinear
    form, so one tiny fp16 matmul per block computes the whole [128 x 48]
    exponent tile:  lhsT = [-INV*x'^2; 2*INV*x'; 1] (device-computed from
    control_points, PE-transposed into sample-major basis tiles), rhs = the
    CONSTANT [1; c'; -INV*c'^2] since columns are recentered at each
    window's center (c'_i = (i-24)/2048 for every window).  Matmul operands
    must start at partition 0/32/64, so basis rows live in 32-partition
    bands (3 blocks per 128-wide PE transpose) and the rhs constant is
    replicated at the three bases.  Exponents for 8 blocks land in one PSUM
    bank; a single ScalarE Exp produces the fp16 Gaussian tile and one
    VectorE multiply applies the host-built exact {0, 2^-8} fp16 mask
    (whose x*y product folds in the 1/65536 normalization).

  The host only mirrors the reference's float32 index math to plan integer
  window origins and the 0/1 masks (scheduling metadata); all float curve
  values are computed on device from the control_points input.  Per-core
  I/O is a handful of resident input DMAs + 1 journal DMA (per-group HWDGE
  descriptor overhead was the previous bottleneck).  The host places the
  32 disjointly-computed per-pair windows of each core into the full grid.
"""
import os
import numpy as np
from contextlib import ExitStack

RES = 2048
STEPS = 65536
SIGMA = 0.01
W = 32
INV = np.float32(1.0 / (2.0 * SIGMA * SIGMA))   # 5000.0
NCORES = 8
SPC = STEPS // NCORES      # samples per core = 8192
NB = SPC // 128            # blocks of 128 samples per core = 64
NP_ = NB // 2              # pairs (two blocks share a window) = 32
NSG = 8                    # supergroups
BPS = NB // NSG            # blocks per supergroup = 8
WIN = 48                   # window width (32 + max drift 16)
NCH0 = 42                  # blocks in transpose chunk 0 (126 basis rows)
RCOL = 128                 # selection-table column offset in the f16 tensor
MCOL = RCOL + NCH0 * WIN   # mask table column offset in the f16 tensor
FCOLS = MCOL + NSG * 768   # f16 const tensor width

LAST_RESULT = None  # BassKernelResults of the last run (for test harness)
LAST_NC = None
LAST_IN_MAPS = None
LAST_METAS = None


# ----------------------------------------------------------------- planning
def _plan(cp: np.ndarray):
    """Host planning: mirrors the reference's float32 index math exactly,
    then builds per-core window origins + fp16 mask tables."""
    p0, p1, p2 = cp[0], cp[1], cp[2]

    # exact mirror of jnp.linspace(0, 1, STEPS, dtype=float32)
    t_lin = np.empty(STEPS, np.float32)
    t_lin[: STEPS - 1] = np.arange(STEPS - 1, dtype=np.float32) / np.float32(
        STEPS - 1
    )
    t_lin[STEPS - 1] = 1.0
    t_out = np.arange(STEPS, dtype=np.float32) / np.float32(STEPS)

    a = p0[:, None] + (p1 - p0)[:, None] * t_lin
    b = p1[:, None] + (p2 - p1)[:, None] * t_lin
    curve = (a + t_out * (b - a)).astype(np.float32)          # [2, S]
    blocks = np.clip(
        np.floor(RES * curve).astype(np.int32) - W // 2, 0, RES - W
    )
    bx, by = blocks[0], blocks[1]

    # device basis tables (float32), pure functions of the step index
    U = (t_lin + t_out).astype(np.float32)
    V = (t_lin * t_out).astype(np.float32)

    # constant column basis: c'_i = (i - 24)/RES (exact in fp16).  The
    # selection table RSEL holds, per block position p within a transpose
    # chunk, a [126 x 48] rhs whose rows are zero except rows 3p..3p+2 =
    # the basis — the zero rows select block p out of the transposed chunk
    # while keeping every matmul operand at partition base 0.
    ci = (np.arange(WIN, dtype=np.float32) - 24.0) / np.float32(RES)
    basis = np.stack([
        np.ones(WIN, np.float32), ci, -INV * ci * ci,
    ]).astype(np.float16)
    rsel = np.zeros((128, NCH0 * WIN), np.float16)
    for p in range(NCH0):
        rsel[3 * p:3 * p + 3, p * WIN:(p + 1) * WIN] = basis

    ident = np.eye(128, dtype=np.float16)

    in_maps = []
    metas = []
    offs = np.arange(WIN, dtype=np.int32)[None, :]
    for c in range(NCORES):
        lo = c * SPC
        bxc = bx[lo: lo + SPC].reshape(NB, 128)
        byc = by[lo: lo + SPC].reshape(NB, 128)

        # per-pair window origins
        ox = np.minimum(bxc.reshape(NP_, 256).min(axis=1), RES - WIN)
        oy = np.minimum(byc.reshape(NP_, 256).min(axis=1), RES - WIN)
        assert (bxc.reshape(NP_, 256).max(axis=1) + W <= ox + WIN).all()
        assert (byc.reshape(NP_, 256).max(axis=1) + W <= oy + WIN).all()

        # masks: value 2^-8 inside the live 32-window (x*y product = 1/65536)
        lox = (bxc - np.repeat(ox, 2)[:, None]).astype(np.int32)  # [NB,128]
        loy = (byc - np.repeat(oy, 2)[:, None]).astype(np.int32)
        mx = ((offs[None] >= lox[:, :, None])
              & (offs[None] < lox[:, :, None] + W))
        my = ((offs[None] >= loy[:, :, None])
              & (offs[None] < loy[:, :, None] + W))
        mx = (mx.astype(np.float16) * np.float16(2.0 ** -8))
        my = (my.astype(np.float16) * np.float16(2.0 ** -8))

        # f16 const tensor: [ident | rsel | per-sg-PAIR (mx 768 | my 768)]
        fct = np.zeros((128, FCOLS), np.float16)
        fct[:, 0:128] = ident
        fct[:, RCOL:MCOL] = rsel
        for sp in range(NSG // 2):
            mb = mx[sp * 16:(sp + 1) * 16]            # [16, 128, WIN]
            yb = my[sp * 16:(sp + 1) * 16]
            s = MCOL + sp * 1536
            fct[:, s: s + 768] = (
                mb.transpose(1, 0, 2).reshape(128, 16 * WIN)
            )
            fct[:, s + 768: s + 1536] = (
                yb.transpose(1, 0, 2).reshape(128, 16 * WIN)
            )

        # window-center tables (f32, exact dyadic)
        ccx = np.repeat((ox + 24).astype(np.float32) / np.float32(RES), 2)
        ccy = np.repeat((oy + 24).astype(np.float32) / np.float32(RES), 2)

        uvc = np.zeros((128, 262), np.float32)
        uvc[:, 0:64] = U[lo: lo + SPC].reshape(NB, 128).T
        uvc[:, 64:128] = V[lo: lo + SPC].reshape(NB, 128).T
        uvc[:, 128:134] = cp.reshape(1, 6).astype(np.float32)
        uvc[:, 134:198] = np.broadcast_to(ccx, (128, NB))
        uvc[:, 198:262] = np.broadcast_to(ccy, (128, NB))

        in_maps.append({"uvc": uvc, "fct": fct})
        metas.append(list(zip(ox.tolist(), oy.tolist())))
    return in_maps, metas


# ------------------------------------------------------------------- device
def _build():
    import concourse.bass as bass
    import concourse.tile as tile
    from concourse import bacc, mybir

    f32 = mybir.dt.float32
    f16 = mybir.dt.float16
    Exp = mybir.ActivationFunctionType.Exp
    mult = mybir.AluOpType.mult
    add = mybir.AluOpType.add
    sub = mybir.AluOpType.subtract

    nc = bacc.Bacc(
        "TRN2", target_bir_lowering=False, debug=False, num_devices=NCORES
    )
    t_uvc = nc.dram_tensor("uvc", [128, 262], f32, kind="ExternalInput").ap()
    t_fct = nc.dram_tensor(
        "fct", [128, FCOLS], f16, kind="ExternalInput"
    ).ap()
    t_out = nc.dram_tensor(
        "out", [112, 2 * 384], f16, kind="ExternalOutput"
    ).ap()

    with tile.TileContext(nc, num_cores=NCORES) as tc, ExitStack() as ctx:
        cpool = ctx.enter_context(tc.tile_pool(name="const", bufs=1))
        sp = ctx.enter_context(tc.tile_pool(name="stream", bufs=2))
        pt = ctx.enter_context(tc.tile_pool(name="psumT", bufs=2,
                                            space="PSUM"))
        pj = ctx.enter_context(tc.tile_pool(name="psumJ", bufs=2,
                                            space="PSUM"))

        uvc = cpool.tile([128, 262], f32, tag="uvc")
        nc.sync.dma_start(uvc[:], t_uvc)
        fct = cpool.tile([128, FCOLS], f16, tag="fct")
        # ident+first-pair rsel land first (they gate the transposes and the
        # first T-matmuls), then the rest of rsel, then masks pair by pair
        cuts = [0, 896, MCOL, MCOL + 1536, MCOL + 2 * 1536,
                MCOL + 3 * 1536, FCOLS]
        for q in range(len(cuts) - 1):
            nc.sync.dma_start(
                fct[:, cuts[q]:cuts[q + 1]], t_fct[:, cuts[q]:cuts[q + 1]]
            )

        # warm up the ScalarE activation table while the DMAs land
        warm = cpool.tile([128, 1], f32, tag="warm")
        nc.vector.memset(warm[:], 0.0)
        warm2 = cpool.tile([128, 1], f16, tag="warm2")
        nc.scalar.activation(warm2[:], warm[:], Exp)

        Ut = uvc[:, 0:64]
        Vt = uvc[:, 64:128]
        cpb = uvc[:, 128:134]
        ccx = uvc[:, 134:198]
        ccy = uvc[:, 198:262]
        ident = fct[:, 0:128]

        # curve coefficients: c1 = p1-p0, c2 = p0-2*p1+p2
        coef = cpool.tile([128, 4], f32, tag="coef")
        nc.vector.tensor_tensor(
            coef[:, 0:2], cpb[:, 2:4], cpb[:, 0:2], op=sub
        )
        nc.vector.scalar_tensor_tensor(
            coef[:, 2:4], cpb[:, 2:4], -2.0, cpb[:, 4:6], op0=mult, op1=add
        )
        nc.vector.tensor_tensor(
            coef[:, 2:4], coef[:, 2:4], cpb[:, 0:2], op=add
        )

        # basis rows packed densely for the PE transpose: PX3[k, b, r]
        PX3 = cpool.tile([128, NB, 3], f16, tag="px3")
        PY3 = cpool.tile([128, NB, 3], f16, tag="py3")

        def axis_basis(eng, c0, c1, c2, cc, P3, tag):
            t1 = cpool.tile([128, NB], f32, tag=f"t1{tag}")
            eng.tensor_scalar(t1[:], Ut, c1, None, op0=mult)
            xw = cpool.tile([128, NB], f32, tag=f"xw{tag}")
            eng.scalar_tensor_tensor(xw[:], Vt, c2, t1[:], op0=mult, op1=add)
            xp = cpool.tile([128, NB], f32, tag=f"xp{tag}")
            eng.scalar_tensor_tensor(xp[:], xw[:], c0, cc, op0=add, op1=sub)
            eng.scalar_tensor_tensor(
                P3[:, :, 0], xp[:], float(-INV), xp[:], op0=mult, op1=mult
            )
            eng.tensor_scalar(
                P3[:, :, 1], xp[:], float(2.0 * INV), None, op0=mult
            )
            eng.memset(P3[:, :, 2], 1.0)

        axis_basis(nc.vector, cpb[:, 0:1], coef[:, 0:1], coef[:, 2:3],
                   ccx, PX3, "x")
        axis_basis(nc.vector, cpb[:, 1:2], coef[:, 1:2], coef[:, 3:4],
                   ccy, PY3, "y")

        # PE-transpose the dense packs (2 chunks per axis: 126 + 66 rows);
        # the T-matmul selects a block's 3 rows via the zero-padded RSEL
        # rhs, so every matmul operand stays at partition base 0
        BC = {}
        with tc.tile_pool(name="ptr", bufs=2, space="PSUM") as ptr:
            ncopy = 0
            for ax, P3 in (("x", PX3), ("y", PY3)):
                tp = ptr.tile([126, 256], f16, tag="tp")
                nc.tensor.transpose(
                    tp[:, 0:128], P3[:, 0:NCH0, :], ident
                )
                nc.tensor.transpose(
                    tp[0:66, 128:256], P3[:, NCH0:NB, :], ident
                )
                bc = cpool.tile([126, 256], f16, tag=f"bc{ax}")
                nc.vector.tensor_copy(bc[:, 0:128], tp[:, 0:128])
                nc.vector.tensor_copy(bc[0:66, 128:256], tp[0:66, 128:256])
                BC[ax] = bc

        # journal PSUM packs two 64-aligned bands of 8 windows per tile; two
        # tiles cover all 32 pair-windows; rows 48:64 of the SBUF journal are
        # never written by the band copies (engine partition accesses must be
        # 32-aligned), so zero them once for the output DMA
        journal = cpool.tile([112, 2 * 384], f16, tag="journal")

        # main loop over supergroup PAIRS: 16 blocks' exponents go into a
        # 2-bank PSUM tile; one Exp / one mask multiply per axis per pair
        JP = None
        for sp_i in range(NSG // 2):
            TX = pt.tile([128, 2, 512], f32, tag="tx", bufs=1)
            TY = pt.tile([128, 2, 512], f32, tag="ty", bufs=1)
            for j2 in range(16):
                b = sp_i * 16 + j2
                pos = b if b < NCH0 else b - NCH0
                kk = 126 if b < NCH0 else 66
                cs = 0 if b < NCH0 else 128
                rb = fct[0:kk, RCOL + pos * WIN: RCOL + (pos + 1) * WIN]
                h, o = j2 // 8, (j2 % 8) * WIN
                nc.tensor.matmul(
                    TX[:, h, o:o + WIN],
                    lhsT=BC["x"][0:kk, cs:cs + 128], rhs=rb,
                    start=True, stop=True,
                )
                nc.tensor.matmul(
                    TY[:, h, o:o + WIN],
                    lhsT=BC["y"][0:kk, cs:cs + 128], rhs=rb,
                    start=True, stop=True,
                )
            EX = sp.tile([128, 768], f16, tag="ex")
            nc.scalar.activation(EX[:], TX[:, :, 0:384], Exp)
            EY = sp.tile([128, 768], f16, tag="ey")
            nc.scalar.activation(EY[:], TY[:, :, 0:384], Exp)
            s = MCOL + sp_i * 1536
            SX = sp.tile([128, 768], f16, tag="sx")
            nc.vector.tensor_tensor(
                SX[:], EX[:], fct[:, s:s + 768], op=mult
            )
            SY = sp.tile([128, 768], f16, tag="sy")
            nc.gpsimd.tensor_tensor(
                SY[:], EY[:], fct[:, s + 768:s + 1536], op=mult
            )
            if sp_i % 2 == 0:
                JP = pj.tile([112, 8 * WIN], f32, tag="jp")
            for j2 in range(16):
                b = sp_i * 16 + j2
                p = b // 2
                band, slot = (p % 16) // 8, p % 8
                nc.tensor.matmul(
                    JP[64 * band:64 * band + WIN,
                       slot * WIN:(slot + 1) * WIN],
                    lhsT=SX[:, j2 * WIN:(j2 + 1) * WIN],
                    rhs=SY[:, j2 * WIN:(j2 + 1) * WIN],
                    start=(b % 2 == 0), stop=(b % 2 == 1),
                )
            # each pair-of-supergroups fills exactly one 48-row band of JP
            t, band = sp_i // 2, sp_i % 2
            r0 = 64 * band
            nc.vector.tensor_copy(
                journal[r0:r0 + WIN, t * 384:(t + 1) * 384],
                JP[r0:r0 + WIN, :],
            )
            nc.sync.dma_start(
                t_out[r0:r0 + WIN, t * 384:(t + 1) * 384],
                journal[r0:r0 + WIN, t * 384:(t + 1) * 384],
            )

    nc.compile()
    return nc


# ------------------------------------------------------------------- driver
def kernel(control_points: np.ndarray) -> np.ndarray:
    global LAST_RESULT, LAST_NC, LAST_IN_MAPS, LAST_METAS
    from concourse.bass_utils import run_bass_kernel_spmd

    cp = np.asarray(control_points, dtype=np.float32)
    in_maps, metas = _plan(cp)
    nc = _build()
    trace = bool(int(os.environ.get("BEZ_TRACE", "0")))
    try:
        res = run_bass_kernel_spmd(
            nc, in_maps, core_ids=list(range(NCORES)), trace=trace
        )
    except ModuleNotFoundError:
        res = run_bass_kernel_spmd(
            nc, in_maps, core_ids=list(range(NCORES)), trace=False
        )
    LAST_RESULT = res
    LAST_NC, LAST_IN_MAPS, LAST_METAS = nc, in_maps, metas

    out = np.zeros((RES, RES), np.float32)
    for c in range(NCORES):
        J = res.results[c]["out"].astype(np.float32)
        for p, (ox, oy) in enumerate(metas[c]):
            t, band, slot = p // 16, (p % 16) // 8, p % 8
            w = J[64 * band:64 * band + WIN,
                  t * 384 + slot * WIN: t * 384 + (slot + 1) * WIN]
            out[ox:ox + WIN, oy:oy + WIN] += w
    return out
